# revision 1
# baseline (speedup 1.0000x reference)
"""Trainium2 Bass kernel for nn_CMAE_8856222564944 (retrieval_knn).

Computation (reference):
    h = L2-normalize rows of x            [B, N_ITEMS]
    h = tanh(h @ W1 + b1)                 [B, 600]
    h = tanh(h @ W2 + b2)                 [B, 200]
    h = tanh(h @ W3 + b3)                 [B, 600]
    dist = |h|^2 - 2 h @ E^T + |E|^2      [B, N_ITEMS]

Distribution (8 cores, tensor-parallel over the item dim):
    - x^T, W1, E^T are sharded over items (rows of W1/x^T, cols of E^T).
    - Each core computes a partial u^T = W1_sh^T x_sh^T; one AllReduce of
      the small [600, B] hidden; the W2/W3 layers are replicated.
    - Each core computes its column shard of dist and the host concatenates.

Precision/throughput strategy:
    - The two big GEMMs (x@W1 and h@E^T) run in fp8-e4m3 with
      perf_mode=DoubleRow (256-deep contraction per PE pass, ~1.5x).
      x is pre-scaled by 128 and W1 by 16 on host (clears e4m3
      subnormals); the 1/2048 is folded into the tanh activation scale.
    - The dist contraction is 602 = 2 DoubleRow k-tiles (h dims 0..511)
      + one bf16 tail tile carrying h dims 512..599 plus the |h|^2, ones
      and |E|^2 aug rows, so the large |h|^2 term stays bf16-accurate.
    - dist is written out as bf16 and upcast on host.
    - MLP (W2/W3) stays bf16; fp32 PSUM accumulation everywhere.
"""

import sys

if "/opt/trn_rl_repo" not in sys.path:
    sys.path.insert(0, "/opt/trn_rl_repo")

import numpy as np
import ml_dtypes

import concourse.bass as bass
import concourse.mybir as mybir
import concourse.tile as tile
from concourse import bacc

BF16 = ml_dtypes.bfloat16
FP8 = ml_dtypes.float8_e4m3
P = 128

# Full-size problem config
N_CORES = 8
B = 1024
H1 = 600
H2 = 200
N_ITEMS = 50000
ITEMS_PAD = 50176          # 8 * 6272, 6272 = 49 * 128
SH = ITEMS_PAD // N_CORES  # per-core item shard
H1P = 608                  # W1 free dim padded so the DoubleRow Ko step is 16B-aligned

X_SCALE = 128.0            # host pre-scale on normalized x before fp8 cast
W1_SCALE = 16.0            # host pre-scale on W1 before fp8 cast
U_SCALE = 1.0 / (X_SCALE * W1_SCALE)


def _chunks(total, size):
    """[(start, length), ...] covering [0, total) in `size` steps."""
    return [(s, min(size, total - s)) for s in range(0, total, size)]


def _dedup_ldweights(nc):
    """Post-scheduling: delete LDWEIGHTS whose stationary operand is identical
    to the previous LDWEIGHTS in the final PE stream (the array still holds
    those weights). Waits carried by a deleted load transfer to the next PE
    instruction so no dependency is lost."""
    import concourse.mybir as mb

    n_skipped = 0
    for bb in nc.main_func.blocks:
        insts = bb.instructions
        prev_key = None
        kill = {}
        for idx, ins in enumerate(insts):
            if getattr(ins, "engine", None) != mb.EngineType.PE:
                continue
            if isinstance(ins, mb.InstLdweights):
                key = (
                    str(ins.ins[0]),
                    ins.perf_mode,
                    ins.is_transpose,
                    ins.tile_position,
                    ins.tile_size,
                )
                if key == prev_key:
                    kill[idx] = ins
                else:
                    prev_key = key
            elif isinstance(ins, (mb.InstMatmult, mb.InstEventSemaphore, mb.InstNoOp)):
                pass  # these leave the loaded weights intact
            else:
                prev_key = None
        if not kill:
            continue
        new_insts = []
        pending = []
        for idx, ins in enumerate(insts):
            if idx in kill:
                pending.append(ins)
                continue
            if pending and getattr(ins, "engine", None) == mb.EngineType.PE:
                for dead in pending:
                    ins.merge_dependencies_from(dead)
                pending = []
            new_insts.append(ins)
        assert not pending
        bb.instructions = new_insts
        n_skipped += len(kill)
    print(f"_dedup_ldweights: removed {n_skipped} redundant weight loads")


def build_program(b=B, h1=H1, h2=H2, sh=SH, n_cores=N_CORES):
    """Build the per-core SPMD Bass program (same graph on every core)."""
    dt = mybir.dt
    fp32 = dt.float32
    bf16 = dt.bfloat16
    fp8 = dt.float8e4
    DR = mybir.MatmulPerfMode.DoubleRow

    assert sh % P == 0
    kdr = sh // 256                   # full DoubleRow item k-tiles (24)
    k_rem = sh - kdr * 256            # leftover rows (128) -> one normal k-tile
    assert k_rem in (0, P)
    mch = _chunks(h1, P)              # H1 row subtiles: [(0,128)..(512,88)]
    m2ch = _chunks(h2, P)             # H2 row subtiles: [(0,128),(128,72)]
    # B column chunks. First chunk 512 wide so its matmul pace hides the W1
    # DMA stream; the trailing 256-chunks give the first chunk's AllReduce
    # ~30us of phase-1 matmuls to hide its (long) doorbell->usable latency.
    bch = [(0, 512), (512, 256), (768, 256)]
    nch = _chunks(sh, 448)            # dist output column tiles
    n_dr = 2                          # DoubleRow k-tiles in dist (h dims 0..511)
    t_rows = h1 - n_dr * 256 + 2      # bf16 tail rows: h 512..599 + hsq + ones

    nc = bacc.Bacc(
        "TRN2",
        target_bir_lowering=False,
        debug=False,
        enable_asserts=False,
        num_devices=n_cores,
    )

    xT = nc.dram_tensor("xT", [sh, b], fp8, kind="ExternalInput")
    W1d = nc.dram_tensor("W1s", [sh, H1P], fp8, kind="ExternalInput")
    W2d = nc.dram_tensor("W2s", [h1, h2], bf16, kind="ExternalInput")
    W3d = nc.dram_tensor("W3s", [h2, h1], bf16, kind="ExternalInput")
    b1d = nc.dram_tensor("b1", [h1], fp32, kind="ExternalInput")
    b2d = nc.dram_tensor("b2", [h2], fp32, kind="ExternalInput")
    b3d = nc.dram_tensor("b3", [h1], fp32, kind="ExternalInput")
    e8d = nc.dram_tensor("e8", [n_dr * 256, sh], fp8, kind="ExternalInput")
    etd = nc.dram_tensor("et", [t_rows, sh], bf16, kind="ExternalInput")
    outd = nc.dram_tensor("dist", [b, sh], bf16, kind="ExternalOutput")

    Tanh = mybir.ActivationFunctionType.Tanh
    Square = mybir.ActivationFunctionType.Square
    rg = [list(range(n_cores))]

    with tile.TileContext(nc) as tc:
        with (
            tc.tile_pool(name="persist", bufs=1) as persist,
            tc.tile_pool(name="dram", bufs=1, space="DRAM") as dram,
            tc.tile_pool(name="psum", bufs=1, space="PSUM") as psum_pool,
            tc.tile_pool(name="outs", bufs=6) as out_pool,
        ):
            # ---- persistent SBUF tensors -------------------------------
            # x^T is SBUF-resident, loaded full-width (1KB descriptors) and
            # consumed by all three B-chunks' k-loops. One tile PER k-tile:
            # slice-writes into a single tile serialize their DMAs (one
            # in-flight transfer), separate tiles stream in parallel.
            x_t = [
                persist.tile([P, 2, b], fp8, name=f"x_t{t}")
                for t in range(kdr + 1)
            ]
            W1_t = [
                persist.tile([P, 2, H1P], fp8, name=f"W1_t{t}")
                for t in range(kdr + 1)
            ]
            e8_sb = persist.tile([P, 2 * n_dr, sh], fp8, name="e8_sb")
            et_sb = persist.tile([t_rows, sh], bf16, name="et_sb")
            W2_sb = persist.tile([P, len(mch), h2], bf16, name="W2_sb")
            W3_sb = persist.tile([P, len(m2ch), h1], bf16, name="W3_sb")
            b1_sb = persist.tile([P, len(mch), 1], fp32, name="b1_sb")
            b2_sb = persist.tile([P, len(m2ch), 1], fp32, name="b2_sb")
            b3_sb = persist.tile([P, len(mch), 1], fp32, name="b3_sb")
            ones_sb = persist.tile([P, len(mch), 1], bf16, name="ones_sb")
            up_sb = persist.tile([P, len(mch), b], bf16, name="up_sb")
            h1_sb = persist.tile([P, len(mch), b], bf16, name="h1_sb")
            h2_sb = persist.tile([P, len(m2ch), b], bf16, name="h2_sb")
            hh16_sb = persist.tile([P, len(mch), b], bf16, name="hh16_sb")
            hh8_sb = persist.tile([P, 2 * n_dr, b], fp8, name="hh8_sb")
            hq_sb = persist.tile([1, b], bf16, name="hq_sb")
            one_row_sb = persist.tile([1, b], bf16, name="one_row_sb")

            nc.vector.memset(ones_sb[:], 1.0)
            nc.vector.memset(one_row_sb[:], 1.0)
            # 'ones' aug row of the dist tail stationary (never overwritten;
            # partition 89 is only reachable by DMA, not compute engines)
            nc.scalar.dma_start(hh16_sb[89:90, len(mch) - 1, :], one_row_sb[0:1, :])

            # ---- phase 1: partial u^T = W1_sh^T @ x_sh^T ----------------
            u_bounce = []
            u_red = []
            for hi, (c0, cl) in enumerate(bch):
                u_bounce.append(
                    dram.tile([h1, cl], bf16, name=f"u_bounce{hi}")
                )
                u_red.append(
                    dram.tile(
                        [h1, cl],
                        bf16,
                        addr_space="Shared" if n_cores > 4 else "Local",
                        name=f"u_red{hi}",
                    )
                )

            # item_emb preload jobs: emitted (on Scalar) between the two
            # phase-1 halves, so their DMAs stream during half 1 — keeping
            # half 0's bandwidth for x/W1 and, critically, keeping their
            # completions out of the rotating-semaphore thresholds that the
            # first AllReduce waits on.
            ech = _chunks(sh, sh // 4)
            e_jobs = [("e8", kk, ec0, ecl) for kk in range(2 * n_dr) for ec0, ecl in ech]
            e_jobs += [("et", 0, ec0, ecl) for ec0, ecl in ech]

            def emit_e_job(ji):
                kind, kk, ec0, ecl = e_jobs[ji]
                if kind == "e8":
                    nc.gpsimd.dma_start(
                        e8_sb[:, kk, ec0 : ec0 + ecl],
                        e8d[kk * P : (kk + 1) * P, ec0 : ec0 + ecl],
                    )
                else:
                    nc.gpsimd.dma_start(
                        et_sb[:, ec0 : ec0 + ecl], etd[:, ec0 : ec0 + ecl]
                    )

            for hi, (c0, cl) in enumerate(bch):
                psums = [
                    psum_pool.tile([P, 512], fp32, name=f"p1_{hi}_{mi}", tag=f"pbank{mi}")
                    for mi in range(len(mch))
                ]
                for t in range(kdr + 1):
                    last = t == kdr
                    if hi == 0:
                        # x and W1 stream once, during chunk 0's k-loop.
                        # x alternates Sync / GpSimd queues (a single queue
                        # tops out ~107GB/s and bounds chunk 0's end, hence
                        # the first AllReduce doorbell); W1 stays on Scalar.
                        xq = nc.sync if t % 2 == 0 else nc.gpsimd
                        if not last:
                            nc.scalar.dma_start(
                                W1_t[t][:, :, :],
                                W1d[256 * t : 256 * t + 256, :].rearrange(
                                    "(o p) m -> p o m", o=2
                                ),
                            )
                            xq.dma_start(
                                x_t[t][:, :, :],
                                xT[256 * t : 256 * t + 256, :].rearrange(
                                    "(o p) c -> p o c", o=2
                                ),
                            )
                        else:
                            nc.scalar.dma_start(
                                W1_t[t][:, 0, :], W1d[256 * kdr :, :]
                            )
                            xq.dma_start(
                                x_t[t][:, 0, :], xT[256 * kdr :, :]
                            )
                    for mi, (m0, ml) in enumerate(mch):
                        if not last:
                            nc.tensor.matmul(
                                psums[mi][:ml, :cl],
                                W1_t[t][:, :, m0 : m0 + ml],
                                x_t[t][:, :, c0 : c0 + cl],
                                start=(t == 0),
                                stop=False,
                                perf_mode=DR,
                            )
                        else:
                            nc.tensor.matmul(
                                psums[mi][:ml, :cl],
                                W1_t[t][:, 0, m0 : m0 + ml],
                                x_t[t][:, 0, c0 : c0 + cl],
                                start=False,
                                stop=True,
                            )
                if hi == 0:
                    # Small et/bias/W2/W3 preloads on GpSimd, emitted after
                    # the x-odd loads (so they don't delay chunk 0's first
                    # tiles) but before the AllReduce — they complete long
                    # before its DMA-counter thresholds are checked.
                    for ji in range(len(e_jobs)):
                        if e_jobs[ji][0] == "et":
                            emit_e_job(ji)
                    for ki, (m0, ml) in enumerate(mch):
                        nc.gpsimd.dma_start(
                            b1_sb[:ml, ki, :],
                            b1d[m0 : m0 + ml].rearrange("(p o) -> p o", o=1),
                        )
                        nc.gpsimd.dma_start(
                            b3_sb[:ml, ki, :],
                            b3d[m0 : m0 + ml].rearrange("(p o) -> p o", o=1),
                        )
                        nc.gpsimd.dma_start(W2_sb[:ml, ki, :], W2d[m0 : m0 + ml, :])
                    for ki, (m0, ml) in enumerate(m2ch):
                        nc.gpsimd.dma_start(
                            b2_sb[:ml, ki, :],
                            b2d[m0 : m0 + ml].rearrange("(p o) -> p o", o=1),
                        )
                        nc.gpsimd.dma_start(W3_sb[:ml, ki, :], W3d[m0 : m0 + ml, :])

                # PSUM evacuation on Vector, off the Scalar queue (which
                # runs the tanh/MLP chain). GpSimd cannot read PSUM.
                # u_bounce DMAs issue from Scalar, NOT Sync: on Sync they
                # queue behind the next half's (pool-backpressured) xt
                # stream and the AllReduce fires a whole half late.
                for mi, (m0, ml) in enumerate(mch):
                    nc.vector.tensor_copy(
                        up_sb[:ml, mi, c0 : c0 + cl], psums[mi][:ml, :cl]
                    )
                    nc.scalar.dma_start(
                        u_bounce[hi][m0 : m0 + ml, :], up_sb[:ml, mi, c0 : c0 + cl]
                    )
                nc.gpsimd.collective_compute(
                    "AllReduce",
                    mybir.AluOpType.add,
                    replica_groups=rg,
                    ins=[u_bounce[hi].opt()],
                    outs=[u_red[hi].opt()],
                )
                if hi == 0:
                    for ji in range(len(e_jobs)):
                        if e_jobs[ji][0] == "e8":
                            emit_e_job(ji)

            # ---- per-B-chunk tail: tanh -> W2 -> W3 -> h_sq -> dist -----
            last_k = len(mch) - 1
            hrow = mch[-1][1]          # h_sq partition within last subtile (88)
            group_sz = 3
            ngroups = [nch[i : i + group_sz] for i in range(0, len(nch), group_sz)]

            def emit_tanh(hi):
                c0, cl = bch[hi]
                for mi, (m0, ml) in enumerate(mch):
                    nc.scalar.dma_start(
                        up_sb[:ml, mi, c0 : c0 + cl], u_red[hi][m0 : m0 + ml, :]
                    )
                    nc.scalar.activation(
                        h1_sb[:ml, mi, c0 : c0 + cl],
                        up_sb[:ml, mi, c0 : c0 + cl],
                        Tanh,
                        bias=b1_sb[:ml, mi, 0:1],
                        scale=U_SCALE,
                    )

            def emit_mlp(hi):
                c0, cl = bch[hi]
                # phase 2 (pbank6/7 — never used by dist)
                for mi, (m0, ml) in enumerate(m2ch):
                    ps = psum_pool.tile([P, 512], fp32, name=f"p2_{hi}_{mi}", tag=f"pbank{6 + mi}")
                    for k, (r0, rl) in enumerate(mch):
                        nc.tensor.matmul(
                            ps[:ml, :cl],
                            W2_sb[:rl, k, m0 : m0 + ml],
                            h1_sb[:rl, k, c0 : c0 + cl],
                            start=(k == 0),
                            stop=(k == len(mch) - 1),
                        )
                    nc.scalar.activation(
                        h2_sb[:ml, mi, c0 : c0 + cl],
                        ps[:ml, :cl],
                        Tanh,
                        bias=b2_sb[:ml, mi, 0:1],
                    )
                # phase 3 (alternates pbank6/7); per m-tile: tanh -> bf16,
                # fp8 convert on Vector, square on Scalar
                for mi, (m0, ml) in enumerate(mch):
                    ps = psum_pool.tile([P, 512], fp32, name=f"p3_{hi}_{mi}", tag=f"pbank{6 + mi % 2}")
                    for k, (r0, rl) in enumerate(m2ch):
                        nc.tensor.matmul(
                            ps[:ml, :cl],
                            W3_sb[:rl, k, m0 : m0 + ml],
                            h2_sb[:rl, k, c0 : c0 + cl],
                            start=(k == 0),
                            stop=(k == len(m2ch) - 1),
                        )
                    nc.scalar.activation(
                        hh16_sb[:ml, mi, c0 : c0 + cl],
                        ps[:ml, :cl],
                        Tanh,
                        bias=b3_sb[:ml, mi, 0:1],
                    )
                    if mi < 2 * 2:  # fp8 copy for the DoubleRow dist k-tiles
                        # half 0: Vector (idle then); half 1: Scalar (Vector
                        # is busy with the half-0 dist evacuations).
                        if hi == 0:
                            nc.vector.tensor_copy(
                                hh8_sb[:ml, mi, c0 : c0 + cl],
                                hh16_sb[:ml, mi, c0 : c0 + cl],
                            )
                        else:
                            nc.scalar.activation(
                                hh8_sb[:ml, mi, c0 : c0 + cl],
                                hh16_sb[:ml, mi, c0 : c0 + cl],
                                mybir.ActivationFunctionType.Copy,
                            )
                    # h^2 into dead h1_sb columns (input to the h_sq matmul)
                    nc.scalar.activation(
                        h1_sb[:ml, mi, c0 : c0 + cl],
                        hh16_sb[:ml, mi, c0 : c0 + cl],
                        Square,
                    )

            def emit_hsq(hi):
                c0, cl = bch[hi]
                psq = psum_pool.tile([1, 512], fp32, name=f"pq_{hi}", tag="pbank6")
                for k, (m0, ml) in enumerate(mch):
                    nc.tensor.matmul(
                        psq[:1, :cl],
                        ones_sb[:ml, k, 0:1],
                        h1_sb[:ml, k, c0 : c0 + cl],
                        start=(k == 0),
                        stop=(k == len(mch) - 1),
                    )
                nc.scalar.copy(hq_sb[0:1, c0 : c0 + cl], psq[:1, :cl])
                # h_sq aug row (partition 88 needs DMA, not compute engines)
                nc.scalar.dma_start(
                    hh16_sb[hrow : hrow + 1, last_k, c0 : c0 + cl],
                    hq_sb[0:1, c0 : c0 + cl],
                )

            def dist_k01(mi, grp, pss):
                for k in range(n_dr):
                    for j, (n0, nl) in enumerate(grp):
                        nc.tensor.matmul(
                            pss[j][:P, :nl],
                            hh8_sb[:, 2 * k : 2 * k + 2, mi * P : (mi + 1) * P],
                            e8_sb[:, 2 * k : 2 * k + 2, n0 : n0 + nl],
                            start=(k == 0),
                            stop=False,
                            perf_mode=DR,
                        )

            def dist_tail(mi, grp, pss):
                for j, (n0, nl) in enumerate(grp):
                    nc.tensor.matmul(
                        pss[j][:P, :nl],
                        hh16_sb[:t_rows, last_k, mi * P : (mi + 1) * P],
                        et_sb[:, n0 : n0 + nl],
                        start=False,
                        stop=True,
                    )

            def dist_evac(mi, grp, pss):
                for j, (n0, nl) in enumerate(grp):
                    ot = out_pool.tile([P, 448], bf16, name=f"ot_{mi}_{j}_{n0}", tag="ot")
                    nc.vector.tensor_copy(ot[:, :nl], pss[j][:P, :nl])
                    nc.sync.dma_start(
                        outd[mi * P : (mi + 1) * P, n0 : n0 + nl], ot[:, :nl]
                    )

            def dist_psums(mi, gi):
                return [
                    psum_pool.tile(
                        [P, 512], fp32, name=f"p4_{mi}_{gi}_{j}",
                        tag=f"pbank{(gi % 2) * 3 + j}",
                    )
                    for j in range(len(ngroups[gi]))
                ]

            def emit_dist(mi_list, hi):
                # First two groups: issue the fp8-DR k-tiles, then the h_sq
                # matmuls, then the tails — hides the square/h_sq/aug-DMA
                # latency under the first ~3us of dist matmuls.
                mi0 = mi_list[0]
                pre = [(mi0, 0, dist_psums(mi0, 0)), (mi0, 1, dist_psums(mi0, 1))]
                for mi, gi, pss in pre:
                    dist_k01(mi, ngroups[gi], pss)
                emit_hsq(hi)
                for mi, gi, pss in pre:
                    dist_tail(mi, ngroups[gi], pss)
                    dist_evac(mi, ngroups[gi], pss)
                for mi in mi_list:
                    for gi, grp in enumerate(ngroups):
                        if mi == mi0 and gi < 2:
                            continue
                        pss = dist_psums(mi, gi)
                        dist_k01(mi, grp, pss)
                        dist_tail(mi, grp, pss)
                        dist_evac(mi, grp, pss)

            half_m = [list(range(c0 // P, (c0 + cl) // P)) for c0, cl in bch]
            for hi in range(len(bch)):
                emit_tanh(hi)
                emit_mlp(hi)
                emit_dist(half_m[hi], hi)

    _dedup_ldweights(nc)
    nc.compile()
    return nc


# ---------------------------------------------------------------------------
# Host side
# ---------------------------------------------------------------------------

def prep_inputs(x, W1, b1, W2, b2, W3, b3, item_emb, n_cores=N_CORES,
                items_pad=ITEMS_PAD):
    """Normalize/cast/transpose/pad/shard the full inputs -> per-core in_maps."""
    n_items = x.shape[1]
    b = x.shape[0]
    h1 = W1.shape[1]
    sh = items_pad // n_cores
    n_dr = 2
    t_rows = h1 - n_dr * 256 + 2

    x = np.asarray(x, np.float32)
    norm = np.sqrt((x * x).sum(axis=1, keepdims=True))
    xn = x / np.maximum(norm, 1e-12)

    xT = np.zeros((items_pad, b), dtype=FP8)
    xT[:n_items] = (xn.T * X_SCALE).astype(FP8)
    W1p = np.zeros((items_pad, H1P), dtype=FP8)
    W1p[:n_items, :h1] = (np.asarray(W1, np.float32) * W1_SCALE).astype(FP8)

    E = np.asarray(item_emb, np.float32)
    m2eT = np.zeros((h1, items_pad), dtype=np.float32)
    m2eT[:, :n_items] = -2.0 * E.T
    e8 = m2eT[: n_dr * 256].astype(FP8)                 # h dims 0..511, fp8
    et = np.zeros((t_rows, items_pad), dtype=BF16)      # bf16 tail
    et[: h1 - n_dr * 256] = m2eT[n_dr * 256 :].astype(BF16)
    et[h1 - n_dr * 256, :] = np.ones((items_pad,), dtype=BF16)
    et[h1 - n_dr * 256 + 1, :n_items] = (E * E).sum(axis=1).astype(BF16)

    common = {
        "W2s": np.ascontiguousarray(np.asarray(W2, np.float32).astype(BF16)),
        "W3s": np.ascontiguousarray(np.asarray(W3, np.float32).astype(BF16)),
        "b1": np.asarray(b1, np.float32),
        "b2": np.asarray(b2, np.float32),
        "b3": np.asarray(b3, np.float32),
    }
    in_maps = []
    for c in range(n_cores):
        in_maps.append(
            dict(
                common,
                xT=np.ascontiguousarray(xT[c * sh : (c + 1) * sh]),
                W1s=np.ascontiguousarray(W1p[c * sh : (c + 1) * sh]),
                e8=np.ascontiguousarray(e8[:, c * sh : (c + 1) * sh]),
                et=np.ascontiguousarray(et[:, c * sh : (c + 1) * sh]),
            )
        )
    return in_maps


_NC_CACHE = {}


def get_nc():
    if "nc" not in _NC_CACHE:
        _NC_CACHE["nc"] = build_program()
    return _NC_CACHE["nc"]


def kernel(x, W1, b1, W2, b2, W3, b3, item_emb, **run_kwargs):
    from concourse.bass_utils import run_bass_kernel_spmd

    n_items = x.shape[1]
    in_maps = prep_inputs(x, W1, b1, W2, b2, W3, b3, item_emb)
    nc = get_nc()
    res = run_bass_kernel_spmd(nc, in_maps, core_ids=list(range(N_CORES)), **run_kwargs)
    dist = np.concatenate(
        [res.results[c]["dist"] for c in range(N_CORES)], axis=1
    )[:, :n_items]
    if run_kwargs:
        kernel.last_results = res
    return np.ascontiguousarray(dist.astype(np.float32))

